# revision 1
# baseline (speedup 1.0000x reference)
"""EntityCrossAttention Trainium2 kernel (bf16-streamed, PE-tiled).

Reference computation (per batch b):
    E = noun_feats[class_ids[b]];  Q = X @ Wq.T + bq;  K,V = E proj
    S = Q @ K.T / sqrt(D);  attn = softmax(S)
    wa = attn * w;  wa /= wa.sum(-1) + 1e-6;  out = wa @ V

Algebra:
  * S = X @ M / sqrt(D) + bias with M = Wq.T @ K.T  [D, N] host-precomputed
    (O(B*N*D^2) total, T-independent).
  * Since sum_n attn_n == 1 exactly, the noun reweighting + renorm is a
    plain softmax with ln(w + 1e-6) folded into the per-(b,n) exp bias
    (up to a negligible 1e-6 * attn * V numerator perturbation):
        out = (e @ V) / (e @ 1),   e = exp(S/sqrt(D) + eb)

Both HBM streams are bf16 (X in, out back): 8 MiB in + 8 MiB out per core
~= the ~358 GB/s/core HBM roofline. Host packs X per 512-row group as
[128, KC*GR] (4 KiB DMA partition lines); out is stored in the grouped
layout and unpacked on host.

Device pipeline per row-group (software-pipelined by one group so the
previous group's den/out matmuls fill the PE while this group's exp runs
on the Scalar engine):
    sc    = M_b.T @ Xt             [N, GR]   PSUM  (KC bf16 matmuls)
    e     = exp(sc*scale + eb)     [N, GR]   SBUF bf16 (one ScalarE act)
    den_a = e_a.T @ ones2          [128, 2]  (NB tiny bf16 matmuls into
                                              col-slices of one PSUM tile)
    rc    = 1/den                  [128, NB] (one batched DVE reciprocal)
    o_a   = (e_a.T @ V) * rc_a     [128, D] -> bf16 SBUF (scale folded
                                    into the PSUM->SBUF copy, ScalarE/DVE
                                    alternating)
All matmul operands are bf16 (fast FWL weight loads); PSUM stays f32.
Sharding: data-parallel over B: 8 cores x 2 batches. X loads on the SP
HWDGE ring; constants + stores on the ACT HWDGE ring.
"""

import numpy as np

B, T, D, C, N = 16, 4096, 512, 14, 32
N_CORES = 8
B_PC = B // N_CORES          # batches per core
ROWS_PC = B_PC * T           # 8192
RT = 128                     # row subtile
GR = 512                     # rows per group
G_PC = ROWS_PC // GR         # 16 groups per core
KC = D // 128                # 4 contraction chunks
NB = GR // RT                # 4 row-subtile bands
SCALE = float(D) ** -0.5

_compiled = None


def _build():
    import concourse.bacc as bacc
    import concourse.tile as tile
    import concourse.mybir as mybir

    f32 = mybir.dt.float32
    bf16 = mybir.dt.bfloat16
    Exp = mybir.ActivationFunctionType.Exp
    Copy = mybir.ActivationFunctionType.Copy

    nc = bacc.Bacc("TRN2", debug=False)
    # x[g*128 + p, k*GR + r] = X[d=k*128+p, row g*GR+r]  (4 KiB lines)
    x = nc.dram_tensor("x", [G_PC * 128, KC * GR], bf16, kind="ExternalInput").ap()
    m = nc.dram_tensor("m", [128, B_PC * KC * N], bf16, kind="ExternalInput").ap()
    # v4[32a + n, b*D + d] = V[b, n, d]   (replicated over the 4 bands)
    v4 = nc.dram_tensor("v4", [128, B_PC * D], bf16, kind="ExternalInput").ap()
    # eb4[32a + n, b] = ebias[b, n]
    eb4 = nc.dram_tensor("eb4", [128, B_PC], f32, kind="ExternalInput").ap()
    one4 = nc.dram_tensor("one4", [128, 2], bf16, kind="ExternalInput").ap()
    # out[g*128 + p, a*D + d] = out_row[g*GR + a*RT + p, d]  (4 KiB lines)
    out = nc.dram_tensor("out", [G_PC * 128, NB * D], bf16,
                         kind="ExternalOutput").ap()

    with tile.TileContext(nc) as tc:
        with (
            tc.tile_pool(name="const", bufs=1) as cpool,
            tc.tile_pool(name="xin", bufs=5) as xpool,
            tc.tile_pool(name="et", bufs=3) as epool,
            tc.tile_pool(name="rcp", bufs=3) as rcpool,
            tc.tile_pool(name="res", bufs=4) as rpool,
            tc.tile_pool(name="ps_sc", bufs=2, space="PSUM") as ps_sc,
            tc.tile_pool(name="ps_den", bufs=2, space="PSUM") as ps_den,
            tc.tile_pool(name="ps_o", bufs=4, space="PSUM") as ps_o,
        ):
            # constants lead the SP ring (~0.3 MB, ~1 us) so the ACT queue
            # holds nothing ahead of the first exp -- an ACT-queue clog here
            # stalls the first den/out matmuls ~3.6 us on the PE
            m_sb = cpool.tile([128, B_PC * KC * N], bf16)
            nc.sync.dma_start(m_sb[:, :], m[:, :])
            eb_sb = cpool.tile([128, B_PC], f32)
            nc.sync.dma_start(eb_sb[:, :], eb4[:, :])
            ones_sb = cpool.tile([128, 2], bf16)
            nc.sync.dma_start(ones_sb[:, :], one4[:, :])
            v_sb = cpool.tile([128, B_PC * D], bf16)
            nc.sync.dma_start(v_sb[:, :], v4[:, :])

            # HAM warm-up: a gapless burst of dummy matmuls in the dead
            # head time (PE would idle waiting for the first X load) to
            # un-throttle the PE clock to 2.4 GHz. Results land in the
            # first scores PSUM buffer and are overwritten by the real
            # scores' start=True.
            sc_warm = ps_sc.tile([N, GR], f32, tag="sc_ps")
            for _ in range(20):
                nc.tensor.matmul(
                    sc_warm[:, 0:256],
                    m_sb[:, 0:N],
                    m_sb[:, 0:256],
                    start=True, stop=True,
                )

            def emit_outs(e2, b, gg, split_store=False):
                den_ps = ps_den.tile([RT, 2 * NB], f32)
                for a in range(NB):
                    nc.tensor.matmul(
                        den_ps[:, 2 * a : 2 * a + 2],
                        e2[:, a * RT : (a + 1) * RT],
                        ones_sb[0:N, :],
                        start=True, stop=True,
                    )
                rc_sb = rcpool.tile([RT, NB], f32)
                nc.vector.reciprocal(rc_sb[:, :], den_ps[:, ::2])

                # 4 row-tiled out matmuls (K=32 bands, concurrent)
                o_ps = []
                for a in range(NB):
                    o_ps_a = ps_o.tile([RT, D], f32, tag="o_ps")
                    o_ps.append(o_ps_a)
                    nc.tensor.matmul(
                        o_ps_a[:, :],
                        e2[:, a * RT : (a + 1) * RT],
                        v_sb[0:N, b * D : (b + 1) * D],
                        start=True, stop=True,
                    )
                o_sb = rpool.tile([RT, NB * D], bf16, tag="o_sb")
                for a in range(NB):
                    dst = o_sb[:, a * D : (a + 1) * D]
                    rc = rc_sb[:, a : a + 1]
                    if a % 2 == 0:
                        nc.scalar.activation(dst, o_ps[a][:, :], Copy, scale=rc)
                    else:
                        nc.vector.tensor_scalar_mul(dst, o_ps[a][:, :], rc)
                    if split_store and a == NB // 2 - 1:
                        # last group: ship the first half early to shorten
                        # the store tail after the final matmuls
                        nc.scalar.dma_start(
                            out[gg * 128 : (gg + 1) * 128, 0 : (NB // 2) * D],
                            o_sb[:, 0 : (NB // 2) * D],
                        )
                if split_store:
                    nc.scalar.dma_start(
                        out[gg * 128 : (gg + 1) * 128, (NB // 2) * D :],
                        o_sb[:, (NB // 2) * D :],
                    )
                else:
                    nc.scalar.dma_start(
                        out[gg * 128 : (gg + 1) * 128, :], o_sb[:, :]
                    )

            prev = None
            for b in range(B_PC):
                for g in range(T // GR):
                    gg = b * (T // GR) + g
                    x_sb = xpool.tile([128, KC * GR], bf16, tag="x_sb")
                    nc.sync.dma_start(
                        x_sb[:, :], x[gg * 128 : (gg + 1) * 128, :]
                    )

                    sc_ps = ps_sc.tile([N, GR], f32)
                    for k in range(KC):
                        nc.tensor.matmul(
                            sc_ps[:, :],
                            m_sb[:, (b * KC + k) * N : (b * KC + k + 1) * N],
                            x_sb[:, k * GR : (k + 1) * GR],
                            start=(k == 0),
                            stop=(k == KC - 1),
                        )
                    e_sb = epool.tile([N, GR], bf16, tag="e_sb")
                    nc.scalar.activation(
                        e_sb[:, :], sc_ps[:, :], Exp,
                        bias=eb_sb[0:N, b : b + 1], scale=SCALE,
                    )
                    e2 = e_sb

                    # previous group's den/out matmuls fill the PE while
                    # this group's exp runs on ACT
                    if prev is not None:
                        emit_outs(*prev)
                    prev = (e2, b, gg)
            emit_outs(*prev, split_store=True)

    nc.compile()
    return nc


def _get_compiled():
    global _compiled
    if _compiled is None:
        _compiled = _build()
    return _compiled


def kernel(
    visual_feat, noun_feats, class_ids, noun_weights,
    Wq, bq, Wk, bk, Wv, bv,
):
    import ml_dtypes
    from concourse.bass_utils import run_bass_kernel_spmd

    bfloat16 = ml_dtypes.bfloat16
    visual_feat = np.asarray(visual_feat, dtype=np.float32)
    noun_feats = np.asarray(noun_feats, dtype=np.float32)
    class_ids = np.asarray(class_ids)
    noun_weights = np.asarray(noun_weights, dtype=np.float32)
    Wq, bq = np.asarray(Wq, np.float32), np.asarray(bq, np.float32)
    Wk, bk = np.asarray(Wk, np.float32), np.asarray(bk, np.float32)
    Wv, bv = np.asarray(Wv, np.float32), np.asarray(bv, np.float32)

    # Host precompute of per-batch constants (all T-independent).
    E = noun_feats[class_ids]                       # [B, N, D]
    W = noun_weights[class_ids]                     # [B, N]
    Kb = E @ Wk.T + bk                              # [B, N, D]
    Vb = E @ Wv.T + bv                              # [B, N, D]
    M = np.einsum("jd,bnj->bdn", Wq, Kb)            # [B, D, N] = Wq.T @ Kb.T
    # exp bias: bq-projection term + ln(w + 1e-6) reweighting fold
    ebias = (Kb @ bq) * SCALE + np.log(W + 1e-6)    # [B, N]

    nc = _get_compiled()

    in_maps = []
    for c in range(N_CORES):
        s = slice(c * B_PC, (c + 1) * B_PC)
        m_c = np.ascontiguousarray(
            M[s].reshape(B_PC, KC, 128, N).transpose(2, 0, 1, 3).reshape(128, -1)
        ).astype(bfloat16)
        # x[g, p, k, r] = Xt[k*128+p, g*GR+r]
        xt_c = visual_feat[s].reshape(ROWS_PC, D).T.astype(bfloat16)
        x_c = np.ascontiguousarray(
            xt_c.reshape(KC, 128, G_PC, GR).transpose(2, 1, 0, 3)
        ).reshape(G_PC * 128, KC * GR)
        v_c = Vb[s].transpose(1, 0, 2).reshape(N, B_PC * D)
        eb_c = ebias[s].T                           # [N, B_PC]
        in_maps.append(
            {
                "x": x_c,
                "m": m_c,
                "v4": np.ascontiguousarray(np.tile(v_c, (128 // N, 1))).astype(bfloat16),
                "eb4": np.ascontiguousarray(np.tile(eb_c, (128 // N, 1))),
                "one4": np.ones((128, 2), np.float32).astype(bfloat16),
            }
        )

    global _last_in_maps
    _last_in_maps = in_maps
    res = run_bass_kernel_spmd(nc, in_maps, list(range(N_CORES)))
    out = np.empty((B, T, D), dtype=np.float32)
    for c in range(N_CORES):
        o = res.results[c]["out"].reshape(G_PC, 128, NB, D)
        o = o.transpose(0, 2, 1, 3).reshape(B_PC, T, D).astype(np.float32)
        out[c * B_PC : (c + 1) * B_PC] = o
    return out



# revision 2
# speedup vs baseline: 1.0826x; 1.0826x over previous
"""EntityCrossAttention Trainium2 kernel (bf16-streamed, PE-tiled).

Reference computation (per batch b):
    E = noun_feats[class_ids[b]];  Q = X @ Wq.T + bq;  K,V = E proj
    S = Q @ K.T / sqrt(D);  attn = softmax(S)
    wa = attn * w;  wa /= wa.sum(-1) + 1e-6;  out = wa @ V

Algebra:
  * S = X @ M / sqrt(D) + bias with M = Wq.T @ K.T  [D, N] host-precomputed
    (O(B*N*D^2) total, T-independent).
  * Since sum_n attn_n == 1 exactly, the noun reweighting + renorm is a
    plain softmax with ln(w + 1e-6) folded into the per-(b,n) exp bias
    (up to a negligible 1e-6 * attn * V numerator perturbation):
        out = (e @ V) / (e @ 1),   e = exp(S/sqrt(D) + eb)

Both HBM streams are bf16 (X in, out back): 8 MiB in + 8 MiB out per core.
The steady-state period is DMA-bound at ~410 GB/s shared read+write
(~2.6 us per 512-row group). Engine budget per group is balanced so every
other engine stays under that:
    PE    : 4 score MMs + 4 den MMs + 4 out MMs + 1 heater  (~2.3 us warm)
    ACT   : exp + 2 PSUM->SBUF scaled copies                (~2.2 us)
    DVE   : 2 scaled copies + reciprocal                    (~1.6 us)
    GPSIMD: const loads (head) + store triggers             (~0.6 us)
    Sync  : X load triggers
The heater matmul (operands: resident SBUF tiles, result overwritten by
the group's real scores via start=True) fills the PE's DMA-wait gap so
the HAM clock gate never sees an idle MID window: without it the PE
re-throttles to 1.2 GHz and the whole kernel runs ~2x slower (the PE at
1.2 GHz becomes the bottleneck, yet never looks busy enough for HAM to
re-warm -- an absorbing cold state).
Sharding: data-parallel over B: 8 cores x 2 batches.
"""

import numpy as np

B, T, D, C, N = 16, 4096, 512, 14, 32
N_CORES = 8
B_PC = B // N_CORES          # batches per core
ROWS_PC = B_PC * T           # 8192
RT = 128                     # row subtile
GR = 512                     # rows per group
G_PC = ROWS_PC // GR         # 16 groups per core
KC = D // 128                # 4 contraction chunks
NB = GR // RT                # 4 row-subtile bands
SCALE = float(D) ** -0.5

_compiled = None


def _build():
    import concourse.bacc as bacc
    import concourse.tile as tile
    import concourse.mybir as mybir

    f32 = mybir.dt.float32
    bf16 = mybir.dt.bfloat16
    Exp = mybir.ActivationFunctionType.Exp
    Copy = mybir.ActivationFunctionType.Copy

    nc = bacc.Bacc("TRN2", debug=False)
    # x[g*128 + p, k*GR + r] = X[d=k*128+p, row g*GR+r]  (4 KiB lines)
    x = nc.dram_tensor("x", [G_PC * 128, KC * GR], bf16, kind="ExternalInput").ap()
    m = nc.dram_tensor("m", [128, B_PC * KC * N], bf16, kind="ExternalInput").ap()
    # v[n, b*D + d] = V[b, n, d]
    v = nc.dram_tensor("v", [N, B_PC * D], bf16, kind="ExternalInput").ap()
    # eb[n, b] = ebias[b, n]
    eb = nc.dram_tensor("eb", [N, B_PC], f32, kind="ExternalInput").ap()
    ones = nc.dram_tensor("ones", [N, 2], bf16, kind="ExternalInput").ap()
    # out[g*128 + p, a*D + d] = out_row[g*GR + a*RT + p, d]  (4 KiB lines)
    out = nc.dram_tensor("out", [G_PC * 128, NB * D], bf16,
                         kind="ExternalOutput").ap()

    with tile.TileContext(nc) as tc:
        with (
            tc.tile_pool(name="const", bufs=1) as cpool,
            tc.tile_pool(name="xin", bufs=5) as xpool,
            tc.tile_pool(name="et", bufs=3) as epool,
            tc.tile_pool(name="rcp", bufs=3) as rcpool,
            tc.tile_pool(name="res", bufs=4) as rpool,
            tc.tile_pool(name="ps_sc", bufs=2, space="PSUM") as ps_sc,
            tc.tile_pool(name="ps_den", bufs=2, space="PSUM") as ps_den,
            tc.tile_pool(name="ps_o", bufs=4, space="PSUM") as ps_o,
        ):
            # Constants ride the GPSIMD HWDGE ring so the Sync ring carries
            # nothing ahead of the first X tile and the ACT ring carries
            # nothing at all (stores also go out on GPSIMD).
            m_sb = cpool.tile([128, B_PC * KC * N], bf16)
            nc.gpsimd.dma_start(m_sb[:, :], m[:, :])
            eb_sb = cpool.tile([N, B_PC], f32)
            nc.gpsimd.dma_start(eb_sb[:, :], eb[:, :])
            ones_sb = cpool.tile([N, 2], bf16)
            nc.gpsimd.dma_start(ones_sb[:, :], ones[:, :])
            v_sb = cpool.tile([N, B_PC * D], bf16)
            nc.gpsimd.dma_start(v_sb[:, :], v[:, :])

            def emit_outs(e2, b, gg, split_store=False):
                den_ps = ps_den.tile([RT, 2 * NB], f32)
                for a in range(NB):
                    nc.tensor.matmul(
                        den_ps[:, 2 * a : 2 * a + 2],
                        e2[:, a * RT : (a + 1) * RT],
                        ones_sb[0:N, :],
                        start=True, stop=True,
                    )
                rc_sb = rcpool.tile([RT, NB], f32)
                nc.vector.reciprocal(rc_sb[:, :], den_ps[:, ::2])

                # 4 row-tiled out matmuls (K=32 bands, concurrent)
                o_ps = []
                for a in range(NB):
                    o_ps_a = ps_o.tile([RT, D], f32, tag="o_ps")
                    o_ps.append(o_ps_a)
                    nc.tensor.matmul(
                        o_ps_a[:, :],
                        e2[:, a * RT : (a + 1) * RT],
                        v_sb[0:N, b * D : (b + 1) * D],
                        start=True, stop=True,
                    )
                o_sb = rpool.tile([RT, NB * D], bf16, tag="o_sb")
                for a in range(NB):
                    dst = o_sb[:, a * D : (a + 1) * D]
                    rc = rc_sb[:, a : a + 1]
                    if a % 2 == 0:
                        nc.scalar.activation(dst, o_ps[a][:, :], Copy, scale=rc)
                    else:
                        nc.vector.tensor_scalar_mul(dst, o_ps[a][:, :], rc)
                    if split_store and a == NB // 2 - 1:
                        # last group: ship the first half early to shorten
                        # the store tail after the final matmuls
                        nc.gpsimd.dma_start(
                            out[gg * 128 : (gg + 1) * 128, 0 : (NB // 2) * D],
                            o_sb[:, 0 : (NB // 2) * D],
                        )
                if split_store:
                    nc.gpsimd.dma_start(
                        out[gg * 128 : (gg + 1) * 128, (NB // 2) * D :],
                        o_sb[:, (NB // 2) * D :],
                    )
                else:
                    nc.gpsimd.dma_start(
                        out[gg * 128 : (gg + 1) * 128, :], o_sb[:, :]
                    )

            prev = None
            prev_x = None
            for b in range(B_PC):
                for g in range(T // GR):
                    gg = b * (T // GR) + g
                    x_sb = xpool.tile([128, KC * GR], bf16, tag="x_sb")
                    nc.sync.dma_start(
                        x_sb[:, :], x[gg * 128 : (gg + 1) * 128, :]
                    )

                    sc_ps = ps_sc.tile([N, GR], f32)
                    # Heater: one dummy matmul on resident operands, issued
                    # ahead of the scores so the PE has work while waiting
                    # for this group's X DMA. Result is discarded (the real
                    # scores start=True reset the PSUM accumulation group).
                    if prev_x is None:
                        nc.tensor.matmul(
                            sc_ps[:, 0:256],
                            m_sb[:, 0:N],
                            m_sb[:, 0:256],
                            start=True, stop=True,
                        )
                    else:
                        nc.tensor.matmul(
                            sc_ps[:, :],
                            m_sb[:, 0:N],
                            prev_x[:, 0:GR],
                            start=True, stop=True,
                        )

                    for k in range(KC):
                        nc.tensor.matmul(
                            sc_ps[:, :],
                            m_sb[:, (b * KC + k) * N : (b * KC + k + 1) * N],
                            x_sb[:, k * GR : (k + 1) * GR],
                            start=(k == 0),
                            stop=(k == KC - 1),
                        )
                    e_sb = epool.tile([N, GR], bf16, tag="e_sb")
                    nc.scalar.activation(
                        e_sb[:, :], sc_ps[:, :], Exp,
                        bias=eb_sb[0:N, b : b + 1], scale=SCALE,
                    )

                    # previous group's den/out matmuls fill the PE while
                    # this group's exp runs on ACT
                    if prev is not None:
                        emit_outs(*prev)
                    prev = (e_sb, b, gg)
                    prev_x = x_sb
            emit_outs(*prev, split_store=True)

    nc.compile()
    return nc


def _get_compiled():
    global _compiled
    if _compiled is None:
        _compiled = _build()
    return _compiled


def kernel(
    visual_feat, noun_feats, class_ids, noun_weights,
    Wq, bq, Wk, bk, Wv, bv,
):
    import ml_dtypes
    from concourse.bass_utils import run_bass_kernel_spmd

    bfloat16 = ml_dtypes.bfloat16
    visual_feat = np.asarray(visual_feat, dtype=np.float32)
    noun_feats = np.asarray(noun_feats, dtype=np.float32)
    class_ids = np.asarray(class_ids)
    noun_weights = np.asarray(noun_weights, dtype=np.float32)
    Wq, bq = np.asarray(Wq, np.float32), np.asarray(bq, np.float32)
    Wk, bk = np.asarray(Wk, np.float32), np.asarray(bk, np.float32)
    Wv, bv = np.asarray(Wv, np.float32), np.asarray(bv, np.float32)

    # Host precompute of per-batch constants (all T-independent).
    E = noun_feats[class_ids]                       # [B, N, D]
    W = noun_weights[class_ids]                     # [B, N]
    Kb = E @ Wk.T + bk                              # [B, N, D]
    Vb = E @ Wv.T + bv                              # [B, N, D]
    M = np.einsum("jd,bnj->bdn", Wq, Kb)            # [B, D, N] = Wq.T @ Kb.T
    # exp bias: bq-projection term + ln(w + 1e-6) reweighting fold
    ebias = (Kb @ bq) * SCALE + np.log(W + 1e-6)    # [B, N]

    nc = _get_compiled()

    in_maps = []
    for c in range(N_CORES):
        s = slice(c * B_PC, (c + 1) * B_PC)
        m_c = np.ascontiguousarray(
            M[s].reshape(B_PC, KC, 128, N).transpose(2, 0, 1, 3).reshape(128, -1)
        ).astype(bfloat16)
        # x[g, p, k, r] = Xt[k*128+p, g*GR+r]
        xt_c = visual_feat[s].reshape(ROWS_PC, D).T.astype(bfloat16)
        x_c = np.ascontiguousarray(
            xt_c.reshape(KC, 128, G_PC, GR).transpose(2, 1, 0, 3)
        ).reshape(G_PC * 128, KC * GR)
        v_c = Vb[s].transpose(1, 0, 2).reshape(N, B_PC * D)
        eb_c = ebias[s].T                           # [N, B_PC]
        in_maps.append(
            {
                "x": x_c,
                "m": m_c,
                "v": np.ascontiguousarray(v_c).astype(bfloat16),
                "eb": np.ascontiguousarray(eb_c),
                "ones": np.ones((N, 2), np.float32).astype(bfloat16),
            }
        )

    global _last_in_maps
    _last_in_maps = in_maps
    res = run_bass_kernel_spmd(nc, in_maps, list(range(N_CORES)))
    out = np.empty((B, T, D), dtype=np.float32)
    for c in range(N_CORES):
        o = res.results[c]["out"].reshape(G_PC, 128, NB, D)
        o = o.transpose(0, 2, 1, 3).reshape(B_PC, T, D).astype(np.float32)
        out[c * B_PC : (c + 1) * B_PC] = o
    return out
